# revision 26
# baseline (speedup 1.0000x reference)
"""Locally-connected conv (LocalLinear) Trainium2 Bass kernel.

Problem: x (B=64, Cin=64, 32, 32), weight (Cout=64, Cin=64, 32, 32, 3, 3),
bias (Cout=64, 32, 32) -> out (B=64, Cout=64, 32, 32).
out[b,o,y,x] = sum_{c,u,v} xpad[b,c,y+u-1,x+v-1] * W[o,c,y,x,u,v] + bias[o,y,x]

Sharding: spatial rows across 8 cores (core i owns output rows y in
[4i, 4i+4) -> 128 locations/core).  Per location it's an independent
64x64 matmul with contraction 576 = Cin*9.

Per-core kernel layout (LoadStationary-balanced union design):
  The PE power-throttles to ~1.2 GHz at high duty, and LoadStationary
  ingests ~2 stationary rows/cycle -- so the PE pace per matmul is
  max(K/2, N) cycles and total LS rows (= weight elements / stationary
  width M) is the dominant PE cost.  To get M=128 despite Cout=64,
  each matmul carries TWO adjacent locations' weights side-by-side in
  the stationary columns (cols 0-63 loc A, 64-127 loc B) over the
  UNION of their tap windows (contraction = (c, window col pair));
  weights outside a location's own window are zero (25% of shipped
  bytes).  6 K=128xM=128xN=64 matmuls per location pair (each exactly
  LS/stream balanced at 64 cycles), one PSUM accumulation group.

  - x lives on SBUF as xs[128, 6, 34, B] fp8e3: partitions 0-63 hold
    xpad[c, 4i+r, xi], partitions 64-127 the same shifted one column
    left (xpad[c, 4i+r, xi+1]).  Moving view xs[0:128, yy+u, xA+2h]
    supplies window cols (xA+2h, xA+2h+1) for both locations at once.
  - weights and x ship as fp8e3 (e3m4: 4 mantissa bits; measured
    1.80e-2 max rel err vs the 2e-2 gate, bit-exact vs host sim since
    the PE dequantizes fp8 exactly into fp32 PSUM): w[128, 64, 6, 128],
    slot s = 2u+h.
  - all inputs stream on the single SP queue in exact consumption
    order (9 DMAs; staying under the framework's ~10 rotating DMA
    semaphore slots avoids issue-side stalls), outputs on the ACT
    queue (5 DMAs, last group split to shorten the tail).
  - drain: one DVE tensor_scalar_add per pair adds bias (per-partition
    scalar) and casts fp32 PSUM -> fp16 SBUF; output returns fp16 and
    the host casts to fp32.
  - measured budget per run: ~7us runtime preamble + ~4.5us first-data
    head + ~23.5us DMA-stream-bound matmul window (8MB input at
    ~350GB/s; PE needs only 10-20us) + ~2.5us output tail + ~8.7us
    framework epilogue.
"""

import numpy as np
import ml_dtypes

import concourse.bacc as bacc
import concourse.mybir as mybir
import concourse.tile as tile
from concourse.bass_utils import run_bass_kernel_spmd

NCORES = 8
B = 64
CIN = 64
COUT = 64
H = 32
ROWS_PER_CORE = H // NCORES  # 4
NJ = 64        # loc-pairs per core (4 yy rows x 16 xp)
JB = 16        # loc-pairs per output DMA group
NBLK = NJ // JB  # 4
WB = 16        # loc-pairs per weight SBUF tile
NWT = NJ // WB  # 4 weight tiles

F16 = mybir.dt.float16
F32 = mybir.dt.float32
F8 = mybir.dt.float8e3
F8_NP = ml_dtypes.float8_e3m4

_nc_cache = None


def _build_nc():
    from contextlib import ExitStack

    nc = bacc.Bacc("TRN2", target_bir_lowering=False)

    w_d = nc.dram_tensor("w", [128, NJ, 6, 128], F8, kind="ExternalInput")
    xs_d = nc.dram_tensor("xs", [128, 6, 34, B], F8, kind="ExternalInput")
    b_d = nc.dram_tensor("bias_p", [128, NJ], F32, kind="ExternalInput")
    o_d = nc.dram_tensor("out_p", [128, NJ, B], F16, kind="ExternalOutput")

    with tile.TileContext(nc) as tc, ExitStack() as ctx:
        xpool = ctx.enter_context(tc.tile_pool(name="xpool", bufs=1))
        wpool = ctx.enter_context(tc.tile_pool(name="wpool", bufs=1))
        bpool = ctx.enter_context(tc.tile_pool(name="bpool", bufs=1))
        opool = ctx.enter_context(tc.tile_pool(name="opool", bufs=4))
        pspool = ctx.enter_context(tc.tile_pool(name="ps", bufs=6, space="PSUM"))

        bias_sb = bpool.tile([128, NJ], F32)

        xs_sb = xpool.tile([128, 6, 34, B], F8)
        w_sbs = [wpool.tile([128, WB, 6, 128], F8, name=f"w_sb{b_}")
                 for b_ in range(NWT)]

        # All inputs stream on the single SP (sync) queue in exact
        # consumption order -- FIFO per queue means bytes arrive in the
        # order the PE needs them, and 8 input DMAs stay within the
        # framework's rotating DMA-semaphore slots (recycling a slot
        # stalls until the previous DMA's consumers all ran).
        def dma_xs_rows(r0, r1, c0=0, c1=34):
            nc.sync.dma_start(xs_sb[:, r0:r1, c0:c1, :],
                              xs_d[:, r0:r1, c0:c1, :])

        def dma_w(blk, j0, j1):
            nc.sync.dma_start(w_sbs[blk][:, j0:j1, :, :],
                              w_d[:, blk * WB + j0:blk * WB + j1, :, :])

        # Coarse chunks at the head (PE start time is not critical --
        # the end is stream-bound), fine chunks at the tail (less PE
        # work left after the last weight byte lands).  bias sits 4th
        # so the output DMA that recycles its semaphore slot is the
        # last one, whose own dependency (the final drains) subsumes
        # the slot's consumer wait.
        dma_xs_rows(0, 3)          # yy=0
        dma_w(0, 0, 8)
        dma_w(0, 8, 16)
        nc.sync.dma_start(bias_sb[:], b_d[:])
        dma_xs_rows(3, 6)          # yy=1..3 (first needed at pair 16)
        dma_w(1, 0, WB)
        dma_w(2, 0, WB)
        dma_w(3, 0, 8)
        dma_w(3, 8, 12)
        dma_w(3, 12, 16)

        for blk in range(NBLK):
            w_sb = w_sbs[blk // (NBLK // NWT)]
            out_sb = opool.tile([128, JB, B], F16)
            for jj in range(JB):
                j = blk * JB + jj
                jw = j % WB
                yy, xp = divmod(j, 16)
                xA = 2 * xp
                ps = pspool.tile([128, B], F32)
                # 6 union matmuls: slot s = 2u+h covers window cols
                # (xA+2h, xA+2h+1) for taps u of both locations.
                for u in range(3):
                    for h in range(2):
                        s = 2 * u + h
                        nc.tensor.matmul(
                            ps[:, :], w_sb[0:128, jw, s, :],
                            xs_sb[0:128, yy + u, xA + 2 * h, :],
                            start=(s == 0), stop=(s == 5),
                            tile_position=(0, 0))
                # Drain: +bias (per-partition scalar), fp32 -> fp16.
                # Alternate DVE/ACT so neither engine's duty trips the
                # power limiter.
                nc.vector.tensor_scalar_add(
                    out_sb[:, jj, :], ps[:], bias_sb[:, j:j + 1])
            if blk < NBLK - 1:
                nc.scalar.dma_start(
                    o_d[:, blk * JB:(blk + 1) * JB, :], out_sb[:])
            else:
                nc.scalar.dma_start(
                    o_d[:, blk * JB:blk * JB + 8, :], out_sb[:, 0:8, :])
                nc.scalar.dma_start(
                    o_d[:, blk * JB + 8:blk * JB + 12, :],
                    out_sb[:, 8:12, :])
                nc.scalar.dma_start(
                    o_d[:, blk * JB + 12:(blk + 1) * JB, :],
                    out_sb[:, 12:JB, :])

    nc.compile()
    return nc


def get_nc():
    global _nc_cache
    if _nc_cache is None:
        _nc_cache = _build_nc()
    return _nc_cache


def prep_inputs(x, weight, bias):
    """Host-side resharding/relayout -> list of 8 per-core input dicts."""
    x = np.asarray(x, dtype=np.float32)
    weight = np.asarray(weight, dtype=np.float32)
    bias = np.asarray(bias, dtype=np.float32)

    # x slices with halo: xs[i, p, r, xi, b]; p<64: xpad[c, 4i+r, xi],
    # p>=64: xpad[c, 4i+r, xi+1] (column-shifted copy for tap pairing).
    xp_ = np.zeros((B, CIN, H + 2, H + 2), np.float32)
    xp_[:, :, 1:H + 1, 1:H + 1] = x
    xs = np.zeros((NCORES, 128, 6, 34, B), F8_NP)
    for i in range(NCORES):
        s = xp_[:, :, 4 * i:4 * i + 6, :].transpose(1, 2, 3, 0)  # (c,6,34,b)
        s8 = s.astype(F8_NP)
        xs[i, 0:64] = s8
        xs[i, 64:128, :, 0:33, :] = s8[:, :, 1:34, :]

    # weights: wp[i, p, j=(yy,xp), s=2u+h, col]; stationary row p = (half,
    # c), half 0 reads window col xA+2h, half 1 reads xA+2h+1; cols 0-63
    # are loc A's output channels, 64-127 loc B's.  Entries where the
    # window col falls outside a location's own 3-tap window are zero.
    T = weight.reshape(COUT, CIN, NCORES, 4, 16, 2, 3, 3)
    T = T.transpose(2, 1, 3, 4, 5, 6, 7, 0)  # i c yy xp xe u v o
    low = np.zeros((NCORES, 64, 4, 16, 3, 2, 128), np.float32)
    high = np.zeros((NCORES, 64, 4, 16, 3, 2, 128), np.float32)
    for u in range(3):
        low[:, :, :, :, u, 0, 0:64] = T[:, :, :, :, 0, u, 0, :]
        low[:, :, :, :, u, 1, 0:64] = T[:, :, :, :, 0, u, 2, :]
        low[:, :, :, :, u, 1, 64:128] = T[:, :, :, :, 1, u, 1, :]
        high[:, :, :, :, u, 0, 0:64] = T[:, :, :, :, 0, u, 1, :]
        high[:, :, :, :, u, 0, 64:128] = T[:, :, :, :, 1, u, 0, :]
        high[:, :, :, :, u, 1, 64:128] = T[:, :, :, :, 1, u, 2, :]
    wp = np.concatenate([low, high], axis=1)  # [i, 128, 4, 16, 3, 2, 128]
    wp = np.ascontiguousarray(
        wp.reshape(NCORES, 128, NJ, 6, 128)).astype(F8_NP)

    # bias: bp[i, p, j]; p<64: bias[p, y, xA], p>=64: bias[p-64, y, xB]
    Bb = bias.reshape(COUT, NCORES, 4, 16, 2)  # o i yy xp xe
    bp = np.ascontiguousarray(
        Bb.transpose(1, 4, 0, 2, 3).reshape(NCORES, 128, NJ), dtype=np.float32)

    return [
        {"w": wp[i],
         "xs": np.ascontiguousarray(xs[i]),
         "bias_p": bp[i]}
        for i in range(NCORES)
    ]


def unpack_output(results):
    """results: list of 8 dicts with 'out_p' [128, NJ, B] -> (B, COUT, H, H)."""
    allout = np.stack([np.asarray(r["out_p"], np.float32) for r in results])
    a = allout.reshape(NCORES, 2, COUT, 4, 16, B)     # i xe o yy xp b
    out = a.transpose(5, 2, 0, 3, 4, 1).reshape(B, COUT, H, H)
    return np.ascontiguousarray(out, dtype=np.float32)


def kernel(x, weight, bias, _trace=False, _tmpdir=None):
    nc = get_nc()
    in_maps = prep_inputs(x, weight, bias)
    res = run_bass_kernel_spmd(
        nc, in_maps, core_ids=list(range(NCORES)),
        trace=_trace, tmpdir=_tmpdir,
        **({"trace_cores": list(range(NCORES))} if _trace else {}),
    )
    out = unpack_output(res.results)
    if _trace:
        kernel.last_results = res
    return out


# revision 28
# speedup vs baseline: 1.0582x; 1.0582x over previous
"""Locally-connected conv (LocalLinear) Trainium2 Bass kernel.

Problem: x (B=64, Cin=64, 32, 32), weight (Cout=64, Cin=64, 32, 32, 3, 3),
bias (Cout=64, 32, 32) -> out (B=64, Cout=64, 32, 32).
out[b,o,y,x] = sum_{c,u,v} xpad[b,c,y+u-1,x+v-1] * W[o,c,y,x,u,v] + bias[o,y,x]

Sharding: spatial rows across 8 cores (core i owns output rows y in
[4i, 4i+4) -> 128 locations/core).  Per location it's an independent
64x64 matmul with contraction 576 = Cin*9.

Per-core kernel layout (LoadStationary-balanced union design):
  The PE power-throttles to ~1.2 GHz at high duty, and LoadStationary
  ingests ~2 stationary rows/cycle -- so the PE pace per matmul is
  max(K/2, N) cycles and total LS rows (= weight elements / stationary
  width M) is the dominant PE cost.  To get M=128 despite Cout=64,
  each matmul carries TWO adjacent locations' weights side-by-side in
  the stationary columns (cols 0-63 loc A, 64-127 loc B) over the
  UNION of their tap windows (contraction = (c, window col pair));
  weights outside a location's own window are zero (25% of shipped
  bytes).  6 K=128xM=128xN=64 matmuls per location pair (each exactly
  LS/stream balanced at 64 cycles), one PSUM accumulation group.

  - x lives on SBUF as xs[128, 6, 34, B] fp8e3: partitions 0-63 hold
    xpad[c, 4i+r, xi], partitions 64-127 the same shifted one column
    left (xpad[c, 4i+r, xi+1]).  Moving view xs[0:128, yy+u, xA+2h]
    supplies window cols (xA+2h, xA+2h+1) for both locations at once.
  - weights and x ship as fp8e3 (e3m4: 4 mantissa bits; measured
    1.80e-2 max rel err vs the 2e-2 gate, bit-exact vs host sim since
    the PE dequantizes fp8 exactly into fp32 PSUM): w[128, 64, 6, 128],
    slot s = 2u+h.
  - all inputs stream on the single SP queue in exact consumption
    order (9 DMAs; staying under the framework's ~10 rotating DMA
    semaphore slots avoids issue-side stalls), outputs on the ACT
    queue (5 DMAs, last group split to shorten the tail).
  - drain: one DVE tensor_scalar_add per pair adds bias (per-partition
    scalar) and casts fp32 PSUM -> fp16 SBUF; output returns fp16 and
    the host casts to fp32.
  - measured budget per run: ~7us runtime preamble + ~4.5us first-data
    head + ~23.5us DMA-stream-bound matmul window (8MB input at
    ~350GB/s; PE needs only 10-20us) + ~2.5us output tail + ~8.7us
    framework epilogue.
"""

import numpy as np
import ml_dtypes

import concourse.bacc as bacc
import concourse.mybir as mybir
import concourse.tile as tile
from concourse.bass_utils import run_bass_kernel_spmd

NCORES = 8
B = 64
CIN = 64
COUT = 64
H = 32
ROWS_PER_CORE = H // NCORES  # 4
NJ = 64        # loc-pairs per core (4 yy rows x 16 xp)
JB = 16        # loc-pairs per output DMA group
NBLK = NJ // JB  # 4
WB = 16        # loc-pairs per weight SBUF tile
NWT = NJ // WB  # 4 weight tiles

F16 = mybir.dt.float16
F32 = mybir.dt.float32
F8 = mybir.dt.float8e3
F8_NP = ml_dtypes.float8_e3m4

_nc_cache = None


def _build_nc():
    from contextlib import ExitStack

    nc = bacc.Bacc("TRN2", target_bir_lowering=False)

    w_d = nc.dram_tensor("w", [128, NJ, 6, 128], F8, kind="ExternalInput")
    xs_d = nc.dram_tensor("xs", [128, 6, 34, B], F8, kind="ExternalInput")
    b_d = nc.dram_tensor("bias_p", [128, NJ], F32, kind="ExternalInput")
    o_d = nc.dram_tensor("out_p", [128, NJ, B], F16, kind="ExternalOutput")

    with tile.TileContext(nc) as tc, ExitStack() as ctx:
        xpool = ctx.enter_context(tc.tile_pool(name="xpool", bufs=1))
        wpool = ctx.enter_context(tc.tile_pool(name="wpool", bufs=1))
        bpool = ctx.enter_context(tc.tile_pool(name="bpool", bufs=1))
        opool = ctx.enter_context(tc.tile_pool(name="opool", bufs=4))
        pspool = ctx.enter_context(tc.tile_pool(name="ps", bufs=6, space="PSUM"))

        bias_sb = bpool.tile([128, NJ], F32)

        xs_sb = xpool.tile([128, 6, 34, B], F8)
        w_sbs = [wpool.tile([128, WB, 6, 128], F8, name=f"w_sb{b_}")
                 for b_ in range(NWT)]

        # All inputs stream on the single SP (sync) queue in exact
        # consumption order -- FIFO per queue means bytes arrive in the
        # order the PE needs them, and 8 input DMAs stay within the
        # framework's rotating DMA-semaphore slots (recycling a slot
        # stalls until the previous DMA's consumers all ran).
        def dma_xs_rows(r0, r1, c0=0, c1=34):
            nc.sync.dma_start(xs_sb[:, r0:r1, c0:c1, :],
                              xs_d[:, r0:r1, c0:c1, :])

        def dma_w(blk, j0, j1):
            nc.sync.dma_start(w_sbs[blk][:, j0:j1, :, :],
                              w_d[:, blk * WB + j0:blk * WB + j1, :, :])

        # Coarse chunks at the head (PE start time is not critical --
        # the end is stream-bound), fine chunks at the tail (less PE
        # work left after the last weight byte lands).  bias sits 4th
        # so the output DMA that recycles its semaphore slot is the
        # last one, whose own dependency (the final drains) subsumes
        # the slot's consumer wait.
        dma_xs_rows(0, 3)          # yy=0
        dma_w(0, 0, 8)
        dma_w(0, 8, 16)
        nc.sync.dma_start(bias_sb[:], b_d[:])
        dma_xs_rows(3, 6)          # yy=1..3 (first needed at pair 16)
        dma_w(1, 0, WB)
        dma_w(2, 0, WB)
        dma_w(3, 0, 8)
        dma_w(3, 8, 12)
        dma_w(3, 12, 16)

        for blk in range(NBLK):
            w_sb = w_sbs[blk // (NBLK // NWT)]
            out_sb = opool.tile([128, JB, B], F16)
            for jj in range(JB):
                j = blk * JB + jj
                jw = j % WB
                yy, xp = divmod(j, 16)
                xA = 2 * xp
                ps = pspool.tile([128, B], F32)
                # 6 union matmuls: slot s = 2u+h covers window cols
                # (xA+2h, xA+2h+1) for taps u of both locations.
                for u in range(3):
                    for h in range(2):
                        s = 2 * u + h
                        nc.tensor.matmul(
                            ps[:, :], w_sb[0:128, jw, s, :],
                            xs_sb[0:128, yy + u, xA + 2 * h, :],
                            start=(s == 0), stop=(s == 5),
                            tile_position=(0, 0))
                # Drain: +bias (per-partition scalar), fp32 -> fp16.
                # Alternate DVE/ACT so neither engine's duty trips the
                # power limiter.
                nc.vector.tensor_scalar_add(
                    out_sb[:, jj, :], ps[:], bias_sb[:, j:j + 1])
            if blk < NBLK - 1:
                nc.scalar.dma_start(
                    o_d[:, blk * JB:(blk + 1) * JB, :], out_sb[:])
            else:
                nc.scalar.dma_start(
                    o_d[:, blk * JB:blk * JB + 8, :], out_sb[:, 0:8, :])
                nc.scalar.dma_start(
                    o_d[:, blk * JB + 8:blk * JB + 12, :],
                    out_sb[:, 8:12, :])
                nc.scalar.dma_start(
                    o_d[:, blk * JB + 12:(blk + 1) * JB, :],
                    out_sb[:, 12:JB, :])

    nc.compile()
    return nc


def get_nc():
    global _nc_cache
    if _nc_cache is None:
        _nc_cache = _build_nc()
    return _nc_cache


def prep_inputs(x, weight, bias):
    """Host-side resharding/relayout -> list of 8 per-core input dicts."""
    x = np.asarray(x, dtype=np.float32)
    weight = np.asarray(weight, dtype=np.float32)
    bias = np.asarray(bias, dtype=np.float32)

    # x slices with halo: xs[i, p, r, xi, b]; p<64: xpad[c, 4i+r, xi],
    # p>=64: xpad[c, 4i+r, xi+1] (column-shifted copy for tap pairing).
    xp_ = np.zeros((B, CIN, H + 2, H + 2), np.float32)
    xp_[:, :, 1:H + 1, 1:H + 1] = x
    xs = np.zeros((NCORES, 128, 6, 34, B), F8_NP)
    for i in range(NCORES):
        s = xp_[:, :, 4 * i:4 * i + 6, :].transpose(1, 2, 3, 0)  # (c,6,34,b)
        s8 = s.astype(F8_NP)
        xs[i, 0:64] = s8
        xs[i, 64:128, :, 0:33, :] = s8[:, :, 1:34, :]

    # weights: wp[i, p, j=(yy,xp), s=2u+h, col]; stationary row p = (half,
    # c), half 0 reads window col xA+2h, half 1 reads xA+2h+1; cols 0-63
    # are loc A's output channels, 64-127 loc B's.  Entries where the
    # window col falls outside a location's own 3-tap window are zero.
    T = weight.reshape(COUT, CIN, NCORES, 4, 16, 2, 3, 3)
    T = T.transpose(2, 1, 3, 4, 5, 6, 7, 0)  # i c yy xp xe u v o
    low = np.zeros((NCORES, 64, 4, 16, 3, 2, 128), np.float32)
    high = np.zeros((NCORES, 64, 4, 16, 3, 2, 128), np.float32)
    for u in range(3):
        low[:, :, :, :, u, 0, 0:64] = T[:, :, :, :, 0, u, 0, :]
        low[:, :, :, :, u, 1, 0:64] = T[:, :, :, :, 0, u, 2, :]
        low[:, :, :, :, u, 1, 64:128] = T[:, :, :, :, 1, u, 1, :]
        high[:, :, :, :, u, 0, 0:64] = T[:, :, :, :, 0, u, 1, :]
        high[:, :, :, :, u, 0, 64:128] = T[:, :, :, :, 1, u, 0, :]
        high[:, :, :, :, u, 1, 64:128] = T[:, :, :, :, 1, u, 2, :]
    wp = np.concatenate([low, high], axis=1)  # [i, 128, 4, 16, 3, 2, 128]
    wp = np.ascontiguousarray(
        wp.reshape(NCORES, 128, NJ, 6, 128)).astype(F8_NP)

    # bias: bp[i, p, j]; p<64: bias[p, y, xA], p>=64: bias[p-64, y, xB]
    Bb = bias.reshape(COUT, NCORES, 4, 16, 2)  # o i yy xp xe
    bp = np.ascontiguousarray(
        Bb.transpose(1, 4, 0, 2, 3).reshape(NCORES, 128, NJ), dtype=np.float32)

    return [
        {"w": wp[i],
         "xs": np.ascontiguousarray(xs[i]),
         "bias_p": bp[i]}
        for i in range(NCORES)
    ]


def unpack_output(results):
    """results: list of 8 dicts with 'out_p' [128, NJ, B] -> (B, COUT, H, H)."""
    allout = np.stack([np.asarray(r["out_p"], np.float32) for r in results])
    a = allout.reshape(NCORES, 2, COUT, 4, 16, B)     # i xe o yy xp b
    out = a.transpose(5, 2, 0, 3, 4, 1).reshape(B, COUT, H, H)
    return np.ascontiguousarray(out, dtype=np.float32)


def kernel(x, weight, bias, _trace=False, _tmpdir=None):
    nc = get_nc()
    in_maps = prep_inputs(x, weight, bias)
    res = run_bass_kernel_spmd(
        nc, in_maps, core_ids=list(range(NCORES)),
        trace=_trace, tmpdir=_tmpdir,
        **({"trace_cores": list(range(NCORES))} if _trace else {}),
    )
    out = unpack_output(res.results)
    if _trace:
        kernel.last_results = res
    return out


# revision 30
# speedup vs baseline: 1.0653x; 1.0067x over previous
"""Locally-connected conv (LocalLinear) Trainium2 Bass kernel.

Problem: x (B=64, Cin=64, 32, 32), weight (Cout=64, Cin=64, 32, 32, 3, 3),
bias (Cout=64, 32, 32) -> out (B=64, Cout=64, 32, 32).
out[b,o,y,x] = sum_{c,u,v} xpad[b,c,y+u-1,x+v-1] * W[o,c,y,x,u,v] + bias[o,y,x]

Sharding: spatial rows across 8 cores (core i owns output rows y in
[4i, 4i+4) -> 128 locations/core).  Per location it's an independent
64x64 matmul with contraction 576 = Cin*9.

Per-core kernel layout (LoadStationary-balanced union design):
  The PE power-throttles to ~1.2 GHz at high duty, and LoadStationary
  ingests ~2 stationary rows/cycle -- so the PE pace per matmul is
  max(K/2, N) cycles and total LS rows (= weight elements / stationary
  width M) is the dominant PE cost.  To get M=128 despite Cout=64,
  each matmul carries TWO adjacent locations' weights side-by-side in
  the stationary columns (cols 0-63 loc A, 64-127 loc B) over the
  UNION of their tap windows (contraction = (c, window col pair));
  weights outside a location's own window are zero (25% of shipped
  bytes).  6 K=128xM=128xN=64 matmuls per location pair (each exactly
  LS/stream balanced at 64 cycles), one PSUM accumulation group.

  - x lives on SBUF as xs[128, 6, 34, B] fp8e3: partitions 0-63 hold
    xpad[c, 4i+r, xi], partitions 64-127 the same shifted one column
    left (xpad[c, 4i+r, xi+1]).  Moving view xs[0:128, yy+u, xA+2h]
    supplies window cols (xA+2h, xA+2h+1) for both locations at once.
  - weights and x ship as fp8e3 (e3m4: 4 mantissa bits; measured
    1.80e-2 max rel err vs the 2e-2 gate, bit-exact vs host sim since
    the PE dequantizes fp8 exactly into fp32 PSUM): w[128, 64, 6, 128],
    slot s = 2u+h.
  - all inputs stream on the single SP queue in exact consumption
    order (9 DMAs; staying under the framework's ~10 rotating DMA
    semaphore slots avoids issue-side stalls), outputs on the ACT
    queue (5 DMAs, last group split to shorten the tail).
  - drain: one DVE tensor_scalar_add per pair adds bias (per-partition
    scalar) and casts fp32 PSUM -> fp16 SBUF; output returns fp16 and
    the host casts to fp32.
  - measured budget per run: ~7us runtime preamble + ~4.5us first-data
    head + ~23.5us DMA-stream-bound matmul window (8MB input at
    ~350GB/s; PE needs only 10-20us) + ~2.5us output tail + ~8.7us
    framework epilogue.
"""

import numpy as np
import ml_dtypes

import concourse.bacc as bacc
import concourse.mybir as mybir
import concourse.tile as tile
from concourse.bass_utils import run_bass_kernel_spmd

NCORES = 8
B = 64
CIN = 64
COUT = 64
H = 32
ROWS_PER_CORE = H // NCORES  # 4
NJ = 64        # loc-pairs per core (4 yy rows x 16 xp)
JB = 16        # loc-pairs per output DMA group
NBLK = NJ // JB  # 4
WB = 16        # loc-pairs per weight SBUF tile
NWT = NJ // WB  # 4 weight tiles

F16 = mybir.dt.float16
F32 = mybir.dt.float32
F8 = mybir.dt.float8e3
F8_NP = ml_dtypes.float8_e3m4

_nc_cache = None


def _build_nc():
    from contextlib import ExitStack

    nc = bacc.Bacc("TRN2", target_bir_lowering=False)

    w_d = nc.dram_tensor("w", [128, NJ, 6, 128], F8, kind="ExternalInput")
    xs_d = nc.dram_tensor("xs", [128, 6, 34, B], F8, kind="ExternalInput")
    b_d = nc.dram_tensor("bias_p", [128, NJ], F32, kind="ExternalInput")
    o_d = nc.dram_tensor("out_p", [128, NJ, B], F16, kind="ExternalOutput")

    with tile.TileContext(nc) as tc, ExitStack() as ctx:
        xpool = ctx.enter_context(tc.tile_pool(name="xpool", bufs=1))
        wpool = ctx.enter_context(tc.tile_pool(name="wpool", bufs=1))
        bpool = ctx.enter_context(tc.tile_pool(name="bpool", bufs=1))
        opool = ctx.enter_context(tc.tile_pool(name="opool", bufs=4))
        pspool = ctx.enter_context(tc.tile_pool(name="ps", bufs=6, space="PSUM"))

        bias_sb = bpool.tile([128, NJ], F32)

        xs_sb = xpool.tile([128, 6, 34, B], F8)
        w_sbs = [wpool.tile([128, WB, 6, 128], F8, name=f"w_sb{b_}")
                 for b_ in range(NWT)]

        # All inputs stream on the single SP (sync) queue in exact
        # consumption order -- FIFO per queue means bytes arrive in the
        # order the PE needs them, and 8 input DMAs stay within the
        # framework's rotating DMA-semaphore slots (recycling a slot
        # stalls until the previous DMA's consumers all ran).
        def dma_xs_rows(r0, r1, c0=0, c1=34):
            nc.sync.dma_start(xs_sb[:, r0:r1, c0:c1, :],
                              xs_d[:, r0:r1, c0:c1, :])

        def dma_w(blk, j0, j1):
            nc.sync.dma_start(w_sbs[blk][:, j0:j1, :, :],
                              w_d[:, blk * WB + j0:blk * WB + j1, :, :])

        # Coarse chunks at the head (PE start time is not critical --
        # the end is stream-bound), fine chunks at the tail (less PE
        # work left after the last weight byte lands).  bias sits 4th
        # so the output DMA that recycles its semaphore slot is the
        # last one, whose own dependency (the final drains) subsumes
        # the slot's consumer wait.
        dma_xs_rows(0, 3)          # yy=0
        dma_w(0, 0, 8)
        dma_w(0, 8, 16)
        nc.sync.dma_start(bias_sb[:], b_d[:])
        dma_xs_rows(3, 6)          # yy=1..3 (first needed at pair 16)
        dma_w(1, 0, WB)
        dma_w(2, 0, WB)
        dma_w(3, 0, 8)
        dma_w(3, 8, 12)
        dma_w(3, 12, 16)

        for blk in range(NBLK):
            w_sb = w_sbs[blk // (NBLK // NWT)]
            out_sb = opool.tile([128, JB, B], F16)
            for jj in range(JB):
                j = blk * JB + jj
                jw = j % WB
                yy, xp = divmod(j, 16)
                xA = 2 * xp
                ps = pspool.tile([128, B], F32)
                # 6 union matmuls: slot s = 2u+h covers window cols
                # (xA+2h, xA+2h+1) for taps u of both locations.
                for u in range(3):
                    for h in range(2):
                        s = 2 * u + h
                        nc.tensor.matmul(
                            ps[:, :], w_sb[0:128, jw, s, :],
                            xs_sb[0:128, yy + u, xA + 2 * h, :],
                            start=(s == 0), stop=(s == 5),
                            tile_position=(0, 0))
                # Drain: +bias (per-partition scalar), fp32 -> fp16.
                # Alternate DVE/ACT so neither engine's duty trips the
                # power limiter.
                nc.vector.tensor_scalar_add(
                    out_sb[:, jj, :], ps[:], bias_sb[:, j:j + 1])
            if blk < NBLK - 1:
                nc.scalar.dma_start(
                    o_d[:, blk * JB:(blk + 1) * JB, :], out_sb[:])
            else:
                nc.scalar.dma_start(
                    o_d[:, blk * JB:blk * JB + 8, :], out_sb[:, 0:8, :])
                nc.scalar.dma_start(
                    o_d[:, blk * JB + 8:blk * JB + 12, :],
                    out_sb[:, 8:12, :])
                nc.scalar.dma_start(
                    o_d[:, blk * JB + 12:(blk + 1) * JB, :],
                    out_sb[:, 12:JB, :])

    nc.compile()
    return nc


def get_nc():
    global _nc_cache
    if _nc_cache is None:
        _nc_cache = _build_nc()
    return _nc_cache


def prep_inputs(x, weight, bias):
    """Host-side resharding/relayout -> list of 8 per-core input dicts."""
    x = np.asarray(x, dtype=np.float32)
    weight = np.asarray(weight, dtype=np.float32)
    bias = np.asarray(bias, dtype=np.float32)

    # x slices with halo: xs[i, p, r, xi, b]; p<64: xpad[c, 4i+r, xi],
    # p>=64: xpad[c, 4i+r, xi+1] (column-shifted copy for tap pairing).
    xp_ = np.zeros((B, CIN, H + 2, H + 2), np.float32)
    xp_[:, :, 1:H + 1, 1:H + 1] = x
    xs = np.zeros((NCORES, 128, 6, 34, B), F8_NP)
    for i in range(NCORES):
        s = xp_[:, :, 4 * i:4 * i + 6, :].transpose(1, 2, 3, 0)  # (c,6,34,b)
        s8 = s.astype(F8_NP)
        xs[i, 0:64] = s8
        xs[i, 64:128, :, 0:33, :] = s8[:, :, 1:34, :]

    # weights: wp[i, p, j=(yy,xp), s=2u+h, col]; stationary row p = (half,
    # c), half 0 reads window col xA+2h, half 1 reads xA+2h+1; cols 0-63
    # are loc A's output channels, 64-127 loc B's.  Entries where the
    # window col falls outside a location's own 3-tap window are zero.
    T = weight.reshape(COUT, CIN, NCORES, 4, 16, 2, 3, 3)
    T = T.transpose(2, 1, 3, 4, 5, 6, 7, 0)  # i c yy xp xe u v o
    low = np.zeros((NCORES, 64, 4, 16, 3, 2, 128), np.float32)
    high = np.zeros((NCORES, 64, 4, 16, 3, 2, 128), np.float32)
    for u in range(3):
        low[:, :, :, :, u, 0, 0:64] = T[:, :, :, :, 0, u, 0, :]
        low[:, :, :, :, u, 1, 0:64] = T[:, :, :, :, 0, u, 2, :]
        low[:, :, :, :, u, 1, 64:128] = T[:, :, :, :, 1, u, 1, :]
        high[:, :, :, :, u, 0, 0:64] = T[:, :, :, :, 0, u, 1, :]
        high[:, :, :, :, u, 0, 64:128] = T[:, :, :, :, 1, u, 0, :]
        high[:, :, :, :, u, 1, 64:128] = T[:, :, :, :, 1, u, 2, :]
    wp = np.concatenate([low, high], axis=1)  # [i, 128, 4, 16, 3, 2, 128]
    wp = np.ascontiguousarray(
        wp.reshape(NCORES, 128, NJ, 6, 128)).astype(F8_NP)

    # bias: bp[i, p, j]; p<64: bias[p, y, xA], p>=64: bias[p-64, y, xB]
    Bb = bias.reshape(COUT, NCORES, 4, 16, 2)  # o i yy xp xe
    bp = np.ascontiguousarray(
        Bb.transpose(1, 4, 0, 2, 3).reshape(NCORES, 128, NJ), dtype=np.float32)

    return [
        {"w": wp[i],
         "xs": np.ascontiguousarray(xs[i]),
         "bias_p": bp[i]}
        for i in range(NCORES)
    ]


def unpack_output(results):
    """results: list of 8 dicts with 'out_p' [128, NJ, B] -> (B, COUT, H, H)."""
    allout = np.stack([np.asarray(r["out_p"], np.float32) for r in results])
    a = allout.reshape(NCORES, 2, COUT, 4, 16, B)     # i xe o yy xp b
    out = a.transpose(5, 2, 0, 3, 4, 1).reshape(B, COUT, H, H)
    return np.ascontiguousarray(out, dtype=np.float32)


def kernel(x, weight, bias, _trace=False, _tmpdir=None):
    nc = get_nc()
    in_maps = prep_inputs(x, weight, bias)
    res = run_bass_kernel_spmd(
        nc, in_maps, core_ids=list(range(NCORES)),
        trace=_trace, tmpdir=_tmpdir,
        **({"trace_cores": list(range(NCORES))} if _trace else {}),
    )
    out = unpack_output(res.results)
    if _trace:
        kernel.last_results = res
    return out


# revision 31
# speedup vs baseline: 1.1305x; 1.0612x over previous
"""Locally-connected conv (LocalLinear) Trainium2 Bass kernel.

Problem: x (B=64, Cin=64, 32, 32), weight (Cout=64, Cin=64, 32, 32, 3, 3),
bias (Cout=64, 32, 32) -> out (B=64, Cout=64, 32, 32).
out[b,o,y,x] = sum_{c,u,v} xpad[b,c,y+u-1,x+v-1] * W[o,c,y,x,u,v] + bias[o,y,x]

Sharding: spatial rows across 8 cores (core i owns output rows y in
[4i, 4i+4) -> 128 locations/core).  Per location it's an independent
64x64 matmul with contraction 576 = Cin*9.

Per-core kernel layout (LoadStationary-balanced union design):
  The PE power-throttles to ~1.2 GHz at high duty, and LoadStationary
  ingests ~2 stationary rows/cycle -- so the PE pace per matmul is
  max(K/2, N) cycles and total LS rows (= weight elements / stationary
  width M) is the dominant PE cost.  To get M=128 despite Cout=64,
  each matmul carries TWO adjacent locations' weights side-by-side in
  the stationary columns (cols 0-63 loc A, 64-127 loc B) over the
  UNION of their tap windows (contraction = (c, window col pair));
  weights outside a location's own window are zero (25% of shipped
  bytes).  6 K=128xM=128xN=64 matmuls per location pair (each exactly
  LS/stream balanced at 64 cycles), one PSUM accumulation group.

  - x lives on SBUF as xs[128, 6, 34, B] fp8e3: partitions 0-63 hold
    xpad[c, 4i+r, xi], partitions 64-127 the same shifted one column
    left (xpad[c, 4i+r, xi+1]).  Moving view xs[0:128, yy+u, xA+2h]
    supplies window cols (xA+2h, xA+2h+1) for both locations at once.
  - weights and x ship as fp8e3 (e3m4: 4 mantissa bits; measured
    1.80e-2 max rel err vs the 2e-2 gate, bit-exact vs host sim since
    the PE dequantizes fp8 exactly into fp32 PSUM): w[128, 64, 6, 128],
    slot s = 2u+h.
  - all inputs stream on the single SP queue in exact consumption
    order (9 DMAs; staying under the framework's ~10 rotating DMA
    semaphore slots avoids issue-side stalls), outputs on the ACT
    queue (5 DMAs, last group split to shorten the tail).
  - drain: one DVE tensor_scalar_add per pair adds bias (per-partition
    scalar) and casts fp32 PSUM -> fp16 SBUF; output returns fp16 and
    the host casts to fp32.
  - measured budget per run: ~7us runtime preamble + ~4.5us first-data
    head + ~23.5us DMA-stream-bound matmul window (8MB input at
    ~350GB/s; PE needs only 10-20us) + ~2.5us output tail + ~8.7us
    framework epilogue.
"""

import numpy as np
import ml_dtypes

import concourse.bacc as bacc
import concourse.mybir as mybir
import concourse.tile as tile
from concourse.bass_utils import run_bass_kernel_spmd

NCORES = 8
B = 64
CIN = 64
COUT = 64
H = 32
ROWS_PER_CORE = H // NCORES  # 4
NJ = 64        # loc-pairs per core (4 yy rows x 16 xp)
JB = 16        # loc-pairs per output DMA group
NBLK = NJ // JB  # 4
WB = 16        # loc-pairs per weight SBUF tile
NWT = NJ // WB  # 4 weight tiles

F16 = mybir.dt.float16
F32 = mybir.dt.float32
F8 = mybir.dt.float8e3
F8_NP = ml_dtypes.float8_e3m4

_nc_cache = None


def _build_nc():
    from contextlib import ExitStack

    nc = bacc.Bacc("TRN2", target_bir_lowering=False)

    w_d = nc.dram_tensor("w", [128, NJ, 6, 128], F8, kind="ExternalInput")
    xs_d = nc.dram_tensor("xs", [128, 6, 34, B], F8, kind="ExternalInput")
    b_d = nc.dram_tensor("bias_p", [128, NJ], F32, kind="ExternalInput")
    o_d = nc.dram_tensor("out_p", [128, NJ, B], F16, kind="ExternalOutput")

    with tile.TileContext(nc) as tc, ExitStack() as ctx:
        xpool = ctx.enter_context(tc.tile_pool(name="xpool", bufs=1))
        wpool = ctx.enter_context(tc.tile_pool(name="wpool", bufs=1))
        bpool = ctx.enter_context(tc.tile_pool(name="bpool", bufs=1))
        opool = ctx.enter_context(tc.tile_pool(name="opool", bufs=4))
        pspool = ctx.enter_context(tc.tile_pool(name="ps", bufs=6, space="PSUM"))

        bias_sb = bpool.tile([128, NJ], F32)

        xs_sb = xpool.tile([128, 6, 34, B], F8)
        w_sbs = [wpool.tile([128, WB, 6, 128], F8, name=f"w_sb{b_}")
                 for b_ in range(NWT)]

        # All inputs stream on the single SP (sync) queue in exact
        # consumption order -- FIFO per queue means bytes arrive in the
        # order the PE needs them, and 8 input DMAs stay within the
        # framework's rotating DMA-semaphore slots (recycling a slot
        # stalls until the previous DMA's consumers all ran).
        def dma_xs_rows(r0, r1, c0=0, c1=34):
            nc.sync.dma_start(xs_sb[:, r0:r1, c0:c1, :],
                              xs_d[:, r0:r1, c0:c1, :])

        def dma_w(blk, j0, j1):
            nc.sync.dma_start(w_sbs[blk][:, j0:j1, :, :],
                              w_d[:, blk * WB + j0:blk * WB + j1, :, :])

        # Coarse chunks at the head (PE start time is not critical --
        # the end is stream-bound), fine chunks at the tail (less PE
        # work left after the last weight byte lands).  bias sits 4th
        # so the output DMA that recycles its semaphore slot is the
        # last one, whose own dependency (the final drains) subsumes
        # the slot's consumer wait.
        dma_xs_rows(0, 3)          # yy=0
        dma_w(0, 0, 8)
        dma_w(0, 8, 16)
        dma_xs_rows(3, 6)          # yy=1..3 (first needed at pair 16)
        dma_w(1, 0, WB)
        dma_w(2, 0, WB)
        dma_w(3, 0, 8)
        dma_w(3, 8, 12)
        dma_w(3, 12, 16)
        # bias emitted last among inputs (on the otherwise-idle ACT
        # queue, so it still transfers immediately): its semaphore slot
        # is consumed by every drain, and this position guarantees no
        # other DMA recycles it.
        nc.scalar.dma_start(bias_sb[:], b_d[:])

        for blk in range(NBLK):
            w_sb = w_sbs[blk // (NBLK // NWT)]
            out_sb = opool.tile([128, JB, B], F16)
            for jj in range(JB):
                j = blk * JB + jj
                jw = j % WB
                yy, xp = divmod(j, 16)
                xA = 2 * xp
                ps = pspool.tile([128, B], F32)
                # 6 union matmuls: slot s = 2u+h covers window cols
                # (xA+2h, xA+2h+1) for taps u of both locations.
                for u in range(3):
                    for h in range(2):
                        s = 2 * u + h
                        nc.tensor.matmul(
                            ps[:, :], w_sb[0:128, jw, s, :],
                            xs_sb[0:128, yy + u, xA + 2 * h, :],
                            start=(s == 0), stop=(s == 5),
                            tile_position=(0, 0))
                # Drain: +bias (per-partition scalar), fp32 -> fp16.
                # Alternate DVE/ACT so neither engine's duty trips the
                # power limiter.
                nc.vector.tensor_scalar_add(
                    out_sb[:, jj, :], ps[:], bias_sb[:, j:j + 1])
            if blk < NBLK - 1:
                nc.sync.dma_start(
                    o_d[:, blk * JB:(blk + 1) * JB, :], out_sb[:])
            else:
                nc.sync.dma_start(
                    o_d[:, blk * JB:blk * JB + 8, :], out_sb[:, 0:8, :])
                nc.sync.dma_start(
                    o_d[:, blk * JB + 8:blk * JB + 12, :],
                    out_sb[:, 8:12, :])
                nc.sync.dma_start(
                    o_d[:, blk * JB + 12:(blk + 1) * JB, :],
                    out_sb[:, 12:JB, :])

    nc.compile()
    return nc


def get_nc():
    global _nc_cache
    if _nc_cache is None:
        _nc_cache = _build_nc()
    return _nc_cache


def prep_inputs(x, weight, bias):
    """Host-side resharding/relayout -> list of 8 per-core input dicts."""
    x = np.asarray(x, dtype=np.float32)
    weight = np.asarray(weight, dtype=np.float32)
    bias = np.asarray(bias, dtype=np.float32)

    # x slices with halo: xs[i, p, r, xi, b]; p<64: xpad[c, 4i+r, xi],
    # p>=64: xpad[c, 4i+r, xi+1] (column-shifted copy for tap pairing).
    xp_ = np.zeros((B, CIN, H + 2, H + 2), np.float32)
    xp_[:, :, 1:H + 1, 1:H + 1] = x
    xs = np.zeros((NCORES, 128, 6, 34, B), F8_NP)
    for i in range(NCORES):
        s = xp_[:, :, 4 * i:4 * i + 6, :].transpose(1, 2, 3, 0)  # (c,6,34,b)
        s8 = s.astype(F8_NP)
        xs[i, 0:64] = s8
        xs[i, 64:128, :, 0:33, :] = s8[:, :, 1:34, :]

    # weights: wp[i, p, j=(yy,xp), s=2u+h, col]; stationary row p = (half,
    # c), half 0 reads window col xA+2h, half 1 reads xA+2h+1; cols 0-63
    # are loc A's output channels, 64-127 loc B's.  Entries where the
    # window col falls outside a location's own 3-tap window are zero.
    T = weight.reshape(COUT, CIN, NCORES, 4, 16, 2, 3, 3)
    T = T.transpose(2, 1, 3, 4, 5, 6, 7, 0)  # i c yy xp xe u v o
    low = np.zeros((NCORES, 64, 4, 16, 3, 2, 128), np.float32)
    high = np.zeros((NCORES, 64, 4, 16, 3, 2, 128), np.float32)
    for u in range(3):
        low[:, :, :, :, u, 0, 0:64] = T[:, :, :, :, 0, u, 0, :]
        low[:, :, :, :, u, 1, 0:64] = T[:, :, :, :, 0, u, 2, :]
        low[:, :, :, :, u, 1, 64:128] = T[:, :, :, :, 1, u, 1, :]
        high[:, :, :, :, u, 0, 0:64] = T[:, :, :, :, 0, u, 1, :]
        high[:, :, :, :, u, 0, 64:128] = T[:, :, :, :, 1, u, 0, :]
        high[:, :, :, :, u, 1, 64:128] = T[:, :, :, :, 1, u, 2, :]
    wp = np.concatenate([low, high], axis=1)  # [i, 128, 4, 16, 3, 2, 128]
    wp = np.ascontiguousarray(
        wp.reshape(NCORES, 128, NJ, 6, 128)).astype(F8_NP)

    # bias: bp[i, p, j]; p<64: bias[p, y, xA], p>=64: bias[p-64, y, xB]
    Bb = bias.reshape(COUT, NCORES, 4, 16, 2)  # o i yy xp xe
    bp = np.ascontiguousarray(
        Bb.transpose(1, 4, 0, 2, 3).reshape(NCORES, 128, NJ), dtype=np.float32)

    return [
        {"w": wp[i],
         "xs": np.ascontiguousarray(xs[i]),
         "bias_p": bp[i]}
        for i in range(NCORES)
    ]


def unpack_output(results):
    """results: list of 8 dicts with 'out_p' [128, NJ, B] -> (B, COUT, H, H)."""
    allout = np.stack([np.asarray(r["out_p"], np.float32) for r in results])
    a = allout.reshape(NCORES, 2, COUT, 4, 16, B)     # i xe o yy xp b
    out = a.transpose(5, 2, 0, 3, 4, 1).reshape(B, COUT, H, H)
    return np.ascontiguousarray(out, dtype=np.float32)


def kernel(x, weight, bias, _trace=False, _tmpdir=None):
    nc = get_nc()
    in_maps = prep_inputs(x, weight, bias)
    res = run_bass_kernel_spmd(
        nc, in_maps, core_ids=list(range(NCORES)),
        trace=_trace, tmpdir=_tmpdir,
        **({"trace_cores": list(range(NCORES))} if _trace else {}),
    )
    out = unpack_output(res.results)
    if _trace:
        kernel.last_results = res
    return out


# revision 33
# speedup vs baseline: 1.1342x; 1.0033x over previous
"""Locally-connected conv (LocalLinear) Trainium2 Bass kernel.

Problem: x (B=64, Cin=64, 32, 32), weight (Cout=64, Cin=64, 32, 32, 3, 3),
bias (Cout=64, 32, 32) -> out (B=64, Cout=64, 32, 32).
out[b,o,y,x] = sum_{c,u,v} xpad[b,c,y+u-1,x+v-1] * W[o,c,y,x,u,v] + bias[o,y,x]

Sharding: spatial rows across 8 cores (core i owns output rows y in
[4i, 4i+4) -> 128 locations/core).  Per location it's an independent
64x64 matmul with contraction 576 = Cin*9.

Per-core kernel layout (LoadStationary-balanced union design):
  The PE power-throttles to ~1.2 GHz at high duty, and LoadStationary
  ingests ~2 stationary rows/cycle -- so the PE pace per matmul is
  max(K/2, N) cycles and total LS rows (= weight elements / stationary
  width M) is the dominant PE cost.  To get M=128 despite Cout=64,
  each matmul carries TWO adjacent locations' weights side-by-side in
  the stationary columns (cols 0-63 loc A, 64-127 loc B) over the
  UNION of their tap windows (contraction = (c, window col pair));
  weights outside a location's own window are zero (25% of shipped
  bytes).  6 K=128xM=128xN=64 matmuls per location pair (each exactly
  LS/stream balanced at 64 cycles), one PSUM accumulation group.

  - x lives on SBUF as xs[128, 6, 34, B] fp8e3: partitions 0-63 hold
    xpad[c, 4i+r, xi], partitions 64-127 the same shifted one column
    left (xpad[c, 4i+r, xi+1]).  Moving view xs[0:128, yy+u, xA+2h]
    supplies window cols (xA+2h, xA+2h+1) for both locations at once.
  - weights and x ship as fp8e3 (e3m4: 4 mantissa bits; measured
    1.80e-2 max rel err vs the 2e-2 gate, bit-exact vs host sim since
    the PE dequantizes fp8 exactly into fp32 PSUM): w[128, 64, 6, 128],
    slot s = 2u+h.
  - inputs AND outputs stream on the single SP queue: inputs in exact
    consumption order, outputs emitted after them so queue FIFO
    guarantees output transfers never preempt the weight stream the PE
    is waiting on (this tightened the 8-core spread to ~1.5us); bias
    rides the idle ACT queue, emitted last so no DMA recycles its
    semaphore slot (whose consumers are all 64 drains).
  - drain: one DVE tensor_scalar_add per pair adds bias (per-partition
    scalar) and casts fp32 PSUM -> fp16 SBUF; output returns fp16 and
    the host casts to fp32.
  - measured budget per run: ~7us runtime preamble + ~4.5us first-data
    head + ~23.5us DMA-stream-bound matmul window (8MB input at
    ~350GB/s; PE needs only 10-20us) + ~2.5us output tail + ~8.7us
    framework epilogue.
"""

import numpy as np
import ml_dtypes

import concourse.bacc as bacc
import concourse.mybir as mybir
import concourse.tile as tile
from concourse.bass_utils import run_bass_kernel_spmd

NCORES = 8
B = 64
CIN = 64
COUT = 64
H = 32
ROWS_PER_CORE = H // NCORES  # 4
NJ = 64        # loc-pairs per core (4 yy rows x 16 xp)
JB = 16        # loc-pairs per output DMA group
NBLK = NJ // JB  # 4
WB = 16        # loc-pairs per weight SBUF tile
NWT = NJ // WB  # 4 weight tiles

F16 = mybir.dt.float16
F32 = mybir.dt.float32
F8 = mybir.dt.float8e3
F8_NP = ml_dtypes.float8_e3m4

_nc_cache = None


def _build_nc():
    from contextlib import ExitStack

    nc = bacc.Bacc("TRN2", target_bir_lowering=False)

    w_d = nc.dram_tensor("w", [128, NJ, 6, 128], F8, kind="ExternalInput")
    xs_d = nc.dram_tensor("xs", [128, 6, 34, B], F8, kind="ExternalInput")
    b_d = nc.dram_tensor("bias_p", [128, NJ], F32, kind="ExternalInput")
    o_d = nc.dram_tensor("out_p", [128, NJ, B], F16, kind="ExternalOutput")

    with tile.TileContext(nc) as tc, ExitStack() as ctx:
        xpool = ctx.enter_context(tc.tile_pool(name="xpool", bufs=1))
        wpool = ctx.enter_context(tc.tile_pool(name="wpool", bufs=1))
        bpool = ctx.enter_context(tc.tile_pool(name="bpool", bufs=1))
        opool = ctx.enter_context(tc.tile_pool(name="opool", bufs=4))
        pspool = ctx.enter_context(tc.tile_pool(name="ps", bufs=8, space="PSUM"))

        bias_sb = bpool.tile([128, NJ], F32)

        xs_sb = xpool.tile([128, 6, 34, B], F8)
        w_sbs = [wpool.tile([128, WB, 6, 128], F8, name=f"w_sb{b_}")
                 for b_ in range(NWT)]

        # All inputs stream on the single SP (sync) queue in exact
        # consumption order -- FIFO per queue means bytes arrive in the
        # order the PE needs them, and 8 input DMAs stay within the
        # framework's rotating DMA-semaphore slots (recycling a slot
        # stalls until the previous DMA's consumers all ran).
        def dma_xs_rows(r0, r1, c0=0, c1=34):
            nc.sync.dma_start(xs_sb[:, r0:r1, c0:c1, :],
                              xs_d[:, r0:r1, c0:c1, :])

        def dma_w(blk, j0, j1):
            nc.sync.dma_start(w_sbs[blk][:, j0:j1, :, :],
                              w_d[:, blk * WB + j0:blk * WB + j1, :, :])

        # Coarse chunks at the head (PE start time is not critical --
        # the end is stream-bound), fine chunks at the tail (less PE
        # work left after the last weight byte lands).
        dma_xs_rows(0, 3)          # yy=0
        dma_w(0, 0, 8)
        dma_w(0, 8, 16)
        dma_xs_rows(3, 6)          # yy=1..3 (first needed at pair 16)
        dma_w(1, 0, WB)
        dma_w(2, 0, WB)
        dma_w(3, 0, 8)
        dma_w(3, 8, 12)
        dma_w(3, 12, 14)
        dma_w(3, 14, 16)
        # bias emitted last among inputs (on the otherwise-idle ACT
        # queue, so it still transfers immediately): its semaphore slot
        # is consumed by every drain, and this position guarantees no
        # other DMA recycles it.
        nc.scalar.dma_start(bias_sb[:], b_d[:])

        for blk in range(NBLK):
            w_sb = w_sbs[blk // (NBLK // NWT)]
            out_sb = opool.tile([128, JB, B], F16)
            for jj in range(JB):
                j = blk * JB + jj
                jw = j % WB
                yy, xp = divmod(j, 16)
                xA = 2 * xp
                ps = pspool.tile([128, B], F32)
                # 6 union matmuls: slot s = 2u+h covers window cols
                # (xA+2h, xA+2h+1) for taps u of both locations.
                for u in range(3):
                    for h in range(2):
                        s = 2 * u + h
                        nc.tensor.matmul(
                            ps[:, :], w_sb[0:128, jw, s, :],
                            xs_sb[0:128, yy + u, xA + 2 * h, :],
                            start=(s == 0), stop=(s == 5),
                            tile_position=(0, 0))
                # Drain: +bias (per-partition scalar), fp32 -> fp16.
                nc.vector.tensor_scalar_add(
                    out_sb[:, jj, :], ps[:], bias_sb[:, j:j + 1])
            if blk < NBLK - 1:
                nc.sync.dma_start(
                    o_d[:, blk * JB:(blk + 1) * JB, :], out_sb[:])
            else:
                nc.sync.dma_start(
                    o_d[:, blk * JB:blk * JB + 8, :], out_sb[:, 0:8, :])
                nc.sync.dma_start(
                    o_d[:, blk * JB + 8:blk * JB + 12, :],
                    out_sb[:, 8:12, :])
                nc.sync.dma_start(
                    o_d[:, blk * JB + 12:(blk + 1) * JB, :],
                    out_sb[:, 12:JB, :])

    nc.compile()
    return nc


def get_nc():
    global _nc_cache
    if _nc_cache is None:
        _nc_cache = _build_nc()
    return _nc_cache


def prep_inputs(x, weight, bias):
    """Host-side resharding/relayout -> list of 8 per-core input dicts."""
    x = np.asarray(x, dtype=np.float32)
    weight = np.asarray(weight, dtype=np.float32)
    bias = np.asarray(bias, dtype=np.float32)

    # x slices with halo: xs[i, p, r, xi, b]; p<64: xpad[c, 4i+r, xi],
    # p>=64: xpad[c, 4i+r, xi+1] (column-shifted copy for tap pairing).
    xp_ = np.zeros((B, CIN, H + 2, H + 2), np.float32)
    xp_[:, :, 1:H + 1, 1:H + 1] = x
    xs = np.zeros((NCORES, 128, 6, 34, B), F8_NP)
    for i in range(NCORES):
        s = xp_[:, :, 4 * i:4 * i + 6, :].transpose(1, 2, 3, 0)  # (c,6,34,b)
        s8 = s.astype(F8_NP)
        xs[i, 0:64] = s8
        xs[i, 64:128, :, 0:33, :] = s8[:, :, 1:34, :]

    # weights: wp[i, p, j=(yy,xp), s=2u+h, col]; stationary row p = (half,
    # c), half 0 reads window col xA+2h, half 1 reads xA+2h+1; cols 0-63
    # are loc A's output channels, 64-127 loc B's.  Entries where the
    # window col falls outside a location's own 3-tap window are zero.
    T = weight.reshape(COUT, CIN, NCORES, 4, 16, 2, 3, 3)
    T = T.transpose(2, 1, 3, 4, 5, 6, 7, 0)  # i c yy xp xe u v o
    low = np.zeros((NCORES, 64, 4, 16, 3, 2, 128), np.float32)
    high = np.zeros((NCORES, 64, 4, 16, 3, 2, 128), np.float32)
    for u in range(3):
        low[:, :, :, :, u, 0, 0:64] = T[:, :, :, :, 0, u, 0, :]
        low[:, :, :, :, u, 1, 0:64] = T[:, :, :, :, 0, u, 2, :]
        low[:, :, :, :, u, 1, 64:128] = T[:, :, :, :, 1, u, 1, :]
        high[:, :, :, :, u, 0, 0:64] = T[:, :, :, :, 0, u, 1, :]
        high[:, :, :, :, u, 0, 64:128] = T[:, :, :, :, 1, u, 0, :]
        high[:, :, :, :, u, 1, 64:128] = T[:, :, :, :, 1, u, 2, :]
    wp = np.concatenate([low, high], axis=1)  # [i, 128, 4, 16, 3, 2, 128]
    wp = np.ascontiguousarray(
        wp.reshape(NCORES, 128, NJ, 6, 128)).astype(F8_NP)

    # bias: bp[i, p, j]; p<64: bias[p, y, xA], p>=64: bias[p-64, y, xB]
    Bb = bias.reshape(COUT, NCORES, 4, 16, 2)  # o i yy xp xe
    bp = np.ascontiguousarray(
        Bb.transpose(1, 4, 0, 2, 3).reshape(NCORES, 128, NJ), dtype=np.float32)

    return [
        {"w": wp[i],
         "xs": np.ascontiguousarray(xs[i]),
         "bias_p": bp[i]}
        for i in range(NCORES)
    ]


def unpack_output(results):
    """results: list of 8 dicts with 'out_p' [128, NJ, B] -> (B, COUT, H, H)."""
    allout = np.stack([np.asarray(r["out_p"], np.float32) for r in results])
    a = allout.reshape(NCORES, 2, COUT, 4, 16, B)     # i xe o yy xp b
    out = a.transpose(5, 2, 0, 3, 4, 1).reshape(B, COUT, H, H)
    return np.ascontiguousarray(out, dtype=np.float32)


def kernel(x, weight, bias, _trace=False, _tmpdir=None):
    nc = get_nc()
    in_maps = prep_inputs(x, weight, bias)
    res = run_bass_kernel_spmd(
        nc, in_maps, core_ids=list(range(NCORES)),
        trace=_trace, tmpdir=_tmpdir,
        **({"trace_cores": list(range(NCORES))} if _trace else {}),
    )
    out = unpack_output(res.results)
    if _trace:
        kernel.last_results = res
    return out
